# revision 21
# baseline (speedup 1.0000x reference)
"""Trainium2 Bass kernel for nn_FDC2_61108794688088.

Math: out[i, c] = BS * s1[i, c] + (W2 @ colsum)[c] + BS * b_fc[c]
  where s1 = z1 @ W_fc[:, :2048].T
        colsum = sum_j relu(z2f @ W_proj.T + b_proj)[j, :]
        W2 = W_fc[:, 2048:]

Sharding: 2D — 4 batch groups x 2 feature halves across the 8 cores.
Core (b, h) handles batch rows [512b, 512b+512) and projection output
features [512h, 512h+512); s1's contraction over the 2048 hidden dims is
split in half between the two h-cores of each batch group (partials summed
on host). Only tiny vectors cross cores, all reduced on host during the
gather: the [1024] colsum and the [65, 512] s1 partials.

The projection matmul runs in fp8 E4M3 (weights pre-scaled by 64, the 1/64
folded into the relu's scale) with DoubleRow packing (256 K-rows per
matmul); s1 runs in float32r (full fp32 precision, 1 cycle/row).

Raw Bacc (no TileContext): everything stays resident in SBUF (no pool
recycling, so no WAR hazards), each projection m-tile owns a PSUM bank,
and ordering is a handful of hand-placed semaphores.
"""

import os
import sys

import numpy as np


def _import_concourse():
    try:
        import concourse.bass  # noqa: F401
    except ImportError:
        for p in ("/opt/trn_rl_repo", "/root/.axon_site/_ro/trn_rl_repo"):
            if os.path.isdir(p) and p not in sys.path:
                sys.path.append(p)
        import concourse.bass  # noqa: F401


_import_concourse()

import ml_dtypes  # noqa: E402

import concourse.bacc as bacc  # noqa: E402
from concourse import mybir  # noqa: E402
from concourse import bass_utils  # noqa: E402

BS = 2048
HID = 2048
PIN = 3 * 56 * 56  # 9408
POUT = 1024
NCLS = 65
NCORES = 8
NB = 4  # batch groups
NH = 2  # feature halves
B = BS // NB  # 512 batch rows per core
MT = POUT // 128 // NH  # 4 m-tiles of 128 output features per core
KT2 = (PIN + 127) // 128  # 74 k-tiles for the projection (padded to 9472)
KP2 = KT2 // 2  # 37 DoubleRow k-pairs
KH = HID // NH  # 1024 hidden dims of s1 contraction per core
KT1 = KH // 128  # 8 k-tiles for s1
WSCALE = 64.0  # fp8 weight pre-scale

FP8 = ml_dtypes.float8_e4m3

_NC_CACHE = None
LAST_RESULTS = None  # BassKernelResults of the most recent run (for profiling)


def _build_nc():
    """Build the per-core Bass module (identical on all 8 cores)."""
    nc = bacc.Bacc(target_bir_lowering=False)
    dt = mybir.dt

    z2ft = nc.dram_tensor("z2ft", [128, KP2, 2, B], dt.float8e4, kind="ExternalInput")
    wpt = nc.dram_tensor(
        "wpt", [MT, 128, KP2, 2, 128], dt.float8e4, kind="ExternalInput"
    )
    bp = nc.dram_tensor("bp", [128, MT], dt.float32, kind="ExternalInput")
    # z1^T block and 2048*W_fc^T block fused into one tensor so the first
    # float32r matmul (self-loading, single sync-wait slot) waits on one DMA.
    zw = nc.dram_tensor("zw", [128, KT1, B + NCLS], dt.float32r, kind="ExternalInput")

    s1t_out = nc.dram_tensor("s1t", [NCLS, B], dt.float32, kind="ExternalOutput")
    colsum_out = nc.dram_tensor("colsum", [128, MT], dt.float32, kind="ExternalOutput")

    # SBUF: everything resident simultaneously (~97 KB/partition of 192).
    z2_sb = nc.alloc_sbuf_tensor("z2_sb", [128, KP2, 2, B], dt.float8e4)[:]
    zw_sb = nc.alloc_sbuf_tensor("zw_sb", [128, KT1, B + NCLS], dt.float32r)[:]
    bp_sb = nc.alloc_sbuf_tensor("bp_sb", [128, MT], dt.float32)[:]
    wp_sb = [
        nc.alloc_sbuf_tensor(f"wp_sb{t}", [128, KP2, 2, 128], dt.float8e4)[:]
        for t in range(MT)
    ]
    relu_sb = nc.alloc_sbuf_tensor("relu_sb", [128, B], dt.float32)[:]
    colsum_sb = nc.alloc_sbuf_tensor("colsum_sb", [128, MT], dt.float32)[:]
    s1_sb = nc.alloc_sbuf_tensor("s1_sb", [NCLS, B], dt.float32)[:]

    # PSUM: one bank per m-tile; s1 reuses bank 0 after act0 consumed it.
    ps = [nc.alloc_psum_tensor(f"ps{t}", [128, B], dt.float32)[:] for t in range(MT)]
    ps1 = ps[0][:NCLS, :]

    # Semaphores: one per input DMA (sync-engine DMAs fan out over several
    # HW queues, so cumulative FIFO thresholds on a shared sem are unsafe).
    s_z2a = nc.alloc_semaphore("s_z2a")
    s_z2b = nc.alloc_semaphore("s_z2b")
    s_bp = nc.alloc_semaphore("s_bp")
    s_zw = nc.alloc_semaphore("s_zw")
    s_wp = [nc.alloc_semaphore(f"s_wp{t}") for t in range(MT)]
    s_wp0b = nc.alloc_semaphore("s_wp0b")
    pesem = nc.alloc_semaphore("pesem")  # +1 per finished psum group
    actsem = nc.alloc_semaphore("actsem")  # +1 per finished activation
    vsem = nc.alloc_semaphore("vsem")  # s1 psum->sbuf copy done
    qout1 = nc.alloc_semaphore("qout1")  # s1t output DMA
    qout2 = nc.alloc_semaphore("qout2")  # colsum output DMA
    donesem = nc.alloc_semaphore("donesem")
    all_sems = (
        [s_z2a, s_z2b, s_bp, s_zw, s_wp0b]
        + s_wp
        + [pesem, actsem, vsem, qout1, qout2, donesem]
    )

    with nc.Block() as block:

        @block.sync
        def _(sync):
            # issue order approximates stream priority; consumption order is
            # m0 (z2+wp0), m1, m2, s1 (zw), m3.
            sync.dma_start(out=z2_sb[:, 0:4], in_=z2ft[:, 0:4]).then_inc(s_z2a, 16)
            sync.dma_start(out=wp_sb[0][:, 0:4], in_=wpt[0, :, 0:4]).then_inc(
                s_wp[0], 16
            )
            sync.dma_start(out=z2_sb[:, 4:KP2], in_=z2ft[:, 4:KP2]).then_inc(
                s_z2b, 16
            )
            sync.dma_start(out=wp_sb[0][:, 4:KP2], in_=wpt[0, :, 4:KP2]).then_inc(
                s_wp0b, 16
            )
            sync.dma_start(out=bp_sb, in_=bp[:]).then_inc(s_bp, 16)
            sync.dma_start(out=wp_sb[1], in_=wpt[1]).then_inc(s_wp[1], 16)
            sync.dma_start(out=wp_sb[2], in_=wpt[2]).then_inc(s_wp[2], 16)
            sync.dma_start(out=zw_sb, in_=zw[:]).then_inc(s_zw, 16)
            sync.dma_start(out=wp_sb[3], in_=wpt[3]).then_inc(s_wp[3], 16)
            # s1 output after the vector copy
            sync.wait_ge(vsem, 1)
            sync.dma_start(out=s1t_out[:], in_=s1_sb).then_inc(qout1, 16)
            sync.wait_ge(qout1, 16)
            sync.sem_inc(donesem, 1)

        @block.tensor
        def _(tensor):
            def proj_tile(t, seg_waits):
                for kp in range(KP2):
                    if kp in seg_waits:
                        for sem, val in seg_waits[kp]:
                            tensor.wait_ge(sem, val)
                    mm = nc.tensor.matmul(
                        ps[t],
                        lhsT=wp_sb[t][:, kp],
                        rhs=z2_sb[:, kp],
                        start=(kp == 0),
                        stop=(kp == KP2 - 1),
                        perf_mode=mybir.MatmulPerfMode.DoubleRow,
                    )
                mm.then_inc(pesem, 1)

            proj_tile(
                0,
                {
                    0: [(s_z2a, 16), (s_wp[0], 16)],
                    4: [(s_z2b, 16), (s_wp0b, 16)],
                },
            )
            proj_tile(1, {0: [(s_wp[1], 16)]})
            proj_tile(2, {0: [(s_wp[2], 16)]})
            # s1 slot: 8 float32r matmuls accumulating 2048*s1^T (K-half);
            # bank-0 psum is free once act0 consumed it.
            tensor.wait_ge(s_zw, 16)
            tensor.wait_ge(actsem, 1)
            for ki in range(KT1):
                mm = nc.tensor.matmul(
                    ps1,
                    lhsT=zw_sb[:, ki, B:],
                    rhs=zw_sb[:, ki, :B],
                    start=(ki == 0),
                    stop=(ki == KT1 - 1),
                )
            mm.then_inc(pesem, 1)
            proj_tile(3, {0: [(s_wp[3], 16)]})

        @block.scalar
        def _(scalar):
            scalar.wait_ge(s_bp, 16)
            # pesem counts: m0,m1,m2 -> 1,2,3, s1 -> 4, m3 -> 5
            thresholds = [1, 2, 3, 5]
            for t in range(MT):
                scalar.wait_ge(pesem, thresholds[t])
                nc.scalar.activation(
                    out=relu_sb,
                    in_=ps[t],
                    func=mybir.ActivationFunctionType.Relu,
                    bias=bp_sb[:, t : t + 1],
                    scale=1.0 / WSCALE,
                    accum_out=colsum_sb[:, t : t + 1],
                ).then_inc(actsem, 1)
            nc.scalar.dma_start(out=colsum_out[:], in_=colsum_sb).then_inc(qout2, 16)
            scalar.wait_ge(qout2, 16)
            scalar.sem_inc(donesem, 1)

        @block.vector
        def _(vector):
            vector.wait_ge(pesem, 4)
            nc.vector.tensor_copy(out=s1_sb, in_=ps1).then_inc(vsem, 1)

        @block.gpsimd
        def _(gpsimd):
            gpsimd.wait_ge(donesem, 2)
            for sem in all_sems:
                gpsimd.sem_clear(sem)

    if not nc.is_finalized():
        nc.finalize()
    return nc


def _prep_inputs(z1, z2, W_proj, b_proj, W_fc):
    """Host-side sharding + layout. Returns per-core input maps.

    Core c = 2*b + h: batch group b (512 rows), feature half h.
    """
    z2f = np.ascontiguousarray(z2.reshape(BS, PIN))

    # z2f^T padded to [74*128, 2048] fp8; per batch group [128, 37, 2, 512]:
    # z2ft[p, t, j, n] = z2f^T[(2t+j)*128 + p, 512b + n]
    Z = np.zeros((KT2 * 128, BS), dtype=FP8)
    Z[:PIN] = z2f.T.astype(FP8)
    z2_shards = [
        np.ascontiguousarray(
            Z[:, b * B : (b + 1) * B].reshape(KP2, 2, 128, B).transpose(2, 0, 1, 3)
        )
        for b in range(NB)
    ]

    # 64 * W_proj^T padded, arranged [8, 128, 37, 2, 128]; half h is the
    # contiguous m-tile slice [4h:4h+4].
    Wp = np.zeros((KT2 * 128, POUT), dtype=FP8)
    Wp[:PIN] = (W_proj.T * np.float32(WSCALE)).astype(FP8)
    wpt_full = np.ascontiguousarray(
        Wp.reshape(KP2, 2, 128, NH * MT, 128).transpose(3, 2, 0, 1, 4)
    )
    wpt_halves = [wpt_full[h * MT : (h + 1) * MT] for h in range(NH)]

    bp_full = b_proj.reshape(NH * MT, 128).T.astype(np.float32)  # [128, 8]
    bp_halves = [
        np.ascontiguousarray(bp_full[:, h * MT : (h + 1) * MT]) for h in range(NH)
    ]

    # 2048 * W_fc[:, :HID]^T split into K-halves [128, 8, 65]
    W1sT = np.float32(BS) * W_fc[:, :HID].T.astype(np.float32)  # [2048, 65]
    w1_halves = [
        np.ascontiguousarray(
            W1sT[h * KH : (h + 1) * KH].reshape(KT1, 128, NCLS).transpose(1, 0, 2)
        )
        for h in range(NH)
    ]

    in_maps = []
    for b in range(NB):
        for h in range(NH):
            z1_blk = (
                z1[b * B : (b + 1) * B, h * KH : (h + 1) * KH]
                .T.reshape(KT1, 128, B)
                .transpose(1, 0, 2)
                .astype(np.float32)
            )
            zw_shard = np.ascontiguousarray(
                np.concatenate([z1_blk, w1_halves[h]], axis=2)
            )
            in_maps.append(
                {
                    "z2ft": z2_shards[b],
                    "wpt": wpt_halves[h],
                    "bp": bp_halves[h],
                    "zw": zw_shard,
                }
            )
    return in_maps


def kernel(z1, z2, W_proj, b_proj, W_fc, b_fc):
    global _NC_CACHE, LAST_RESULTS

    z1 = np.asarray(z1, dtype=np.float32)
    z2 = np.asarray(z2, dtype=np.float32)
    W_proj = np.asarray(W_proj, dtype=np.float32)
    b_proj = np.asarray(b_proj, dtype=np.float32)
    W_fc = np.asarray(W_fc, dtype=np.float32)
    b_fc = np.asarray(b_fc, dtype=np.float32)

    if _NC_CACHE is None:
        _NC_CACHE = _build_nc()
    nc = _NC_CACHE

    in_maps = _prep_inputs(z1, z2, W_proj, b_proj, W_fc)
    res = bass_utils.run_bass_kernel_spmd(nc, in_maps, core_ids=list(range(NCORES)))
    LAST_RESULTS = res

    # gather: sum the two K-half s1 partials per batch group, concat groups;
    # colsum chunks concat over h after summing over b.
    A = np.concatenate(
        [
            (
                np.asarray(res.results[2 * b]["s1t"]).astype(np.float64)
                + np.asarray(res.results[2 * b + 1]["s1t"]).astype(np.float64)
            ).T
            for b in range(NB)
        ],
        axis=0,
    )  # [2048, 65], already scaled by BS
    colsum = np.zeros(POUT, dtype=np.float64)
    for h in range(NH):
        acc = np.zeros((128, MT), dtype=np.float64)
        for b in range(NB):
            acc += np.asarray(res.results[2 * b + h]["colsum"]).astype(np.float64)
        colsum[h * MT * 128 : (h + 1) * MT * 128] = acc.T.reshape(-1)
    vec = W_fc[:, HID:].astype(np.float64) @ colsum + np.float64(BS) * b_fc.astype(
        np.float64
    )
    out = A + vec[None, :]
    return out.astype(np.float32)
